# revision 5
# baseline (speedup 1.0000x reference)
"""GCN layer (COO SpMM + linear) on 8 Trainium2 NeuronCores.

Strategy (per sharding hint): shard destination nodes across the 8 cores
(12,500 rows each); partition edges by destination so the segment-sum is
core-local; replicate X (each core gathers source rows from its own full
copy in HBM) and the small [128,128] weight.

Per-core kernel:
  - dest nodes are grouped into blocks of 128 (one PSUM accumulator per
    block), blocks into super-blocks of SBLK (bounded PSUM pressure).
  - dma_gather uses int16 indices, so X is windowed into chunks of 32768
    rows; edges are bucketed per (block, chunk) cell and each cell padded
    to whole 128-edge batches. The batch schedule (super-block -> chunk ->
    block) is shared across cores (max over cores per cell) so one SPMD
    program serves all 8 cores.
  - per batch (128 edge slots, one per SBUF partition):
      Xg   = dma_gather of X[col[e]] rows            [128e, 128f]
      S_T  = val[e] * (iota[d] == dloc[e])   (one fused DVE tensor_scalar)
      h.T += Xg.T @ S_T   (PE matmul, PSUM accumulate over the block)
  - per block: y = (h.T).T @ W.T + b via a second matmul, staged per
    super-block and DMA'd out.
"""

import sys

import numpy as np

sys.path.insert(0, "/opt/trn_rl_repo")

import concourse.bacc as bacc
import concourse.mybir as mybir
import concourse.tile as tile
from concourse.bass_utils import run_bass_kernel_spmd

N_NODES = 100000
D = 128
N_CORES = 8
NPC = N_NODES // N_CORES  # nodes per core
P = 128
CHUNK = 32768  # int16 index window over X rows
SBLK = 6  # blocks per super-block (PSUM accumulators alive)

F32 = mybir.dt.float32
I16 = mybir.dt.int16


def _chunk_bounds(n_nodes, chunk):
    ch = list(range(0, n_nodes, chunk)) + [n_nodes]
    return np.array(ch, dtype=np.int64)


def _schedule(counts, sblk):
    """counts: [n_cores, nb, nq] -> shared batch schedule."""
    nb, nq = counts.shape[1], counts.shape[2]
    K = -(-counts.max(axis=0) // P)  # [nb, nq] ceil
    for b in range(nb):
        if K[b].sum() == 0:
            K[b, 0] = 1
    batches = []  # (b, q) per batch
    runs = []  # (q, t0, R) per gather run
    for u in range(0, nb, sblk):
        blocks = range(u, min(u + sblk, nb))
        for q in range(nq):
            t0 = len(batches)
            for b in blocks:
                batches += [(b, q)] * int(K[b, q])
            r = len(batches) - t0
            if r:
                runs.append((q, t0, r))
    T = len(batches)
    first, last = {}, {}
    for t, (b, q) in enumerate(batches):
        first.setdefault(b, t)
        last[b] = t
    cell_t0 = np.zeros((nb, nq), dtype=np.int64)
    seen = set()
    for t, (b, q) in enumerate(batches):
        if (b, q) not in seen:
            cell_t0[b, q] = t
            seen.add((b, q))
    return K, batches, runs, first, last, cell_t0, T


def _prep(A_rows, A_cols, A_vals, n_cores, npc, ch, sblk):
    nb = (npc + P - 1) // P
    nq = len(ch) - 1
    core = A_rows // npc
    rl = A_rows - core * npc
    blk = rl // P
    q = np.searchsorted(ch, A_cols, side="right") - 1
    cell = (core * nb + blk) * nq + q
    counts = np.bincount(cell, minlength=n_cores * nb * nq).reshape(
        n_cores, nb, nq
    )
    K, batches, runs, first, last, cell_t0, T = _schedule(counts, sblk)
    metas = []
    for c in range(n_cores):
        m = core == c
        rl_c, cols_c, vals_c = rl[m], A_cols[m], A_vals[m]
        cell_c = blk[m] * nq + q[m]
        order = np.argsort(cell_c, kind="stable")
        rl_c, cols_c, vals_c, cell_c = (
            rl_c[order],
            cols_c[order],
            vals_c[order],
            cell_c[order],
        )
        ccounts = counts[c].reshape(-1)
        starts = np.concatenate([[0], np.cumsum(ccounts)])[:-1]
        pos = np.arange(rl_c.size) - starts[cell_c]
        slot = cell_t0.reshape(-1)[cell_c] * P + pos
        t_of = slot // P
        i_of = slot % P
        idx16 = (cols_c - ch[q[m][order]]).astype(np.int16)
        idx_flat = np.zeros((16, 8 * T), np.int16)
        idx_flat[i_of % 16, t_of * 8 + i_of // 16] = idx16
        idx_all = np.tile(idx_flat, (8, 1))
        dloc_t = np.zeros((P, T), np.float32)
        val_t = np.zeros((P, T), np.float32)
        dloc_t[i_of, t_of] = (rl_c % P).astype(np.float32)
        val_t[i_of, t_of] = vals_c
        metas.append((idx_all, dloc_t, val_t))
    return metas, (K, batches, runs, first, last, T), nb, nq


def _build_program(n_nodes, ch, sched, nb, sblk):
    K, batches, runs, first, last, T = sched
    nc = bacc.Bacc(
        "TRN2", target_bir_lowering=False, debug=False, num_devices=N_CORES
    )
    x_d = nc.dram_tensor("X", [n_nodes, D], F32, kind="ExternalInput").ap()
    idx_d = nc.dram_tensor("idx", [P, 8 * T], I16, kind="ExternalInput").ap()
    dloc_d = nc.dram_tensor("dloc", [P, T], F32, kind="ExternalInput").ap()
    val_d = nc.dram_tensor("val", [P, T], F32, kind="ExternalInput").ap()
    wt_d = nc.dram_tensor("wt", [P, D], F32, kind="ExternalInput").ap()
    bb_d = nc.dram_tensor("bb", [P, D], F32, kind="ExternalInput").ap()
    iota_d = nc.dram_tensor("iota", [P, P], F32, kind="ExternalInput").ap()
    y_d = nc.dram_tensor("y", [nb * P, D], F32, kind="ExternalOutput").ap()

    with tile.TileContext(nc) as tc:
        with (
            tc.tile_pool(name="const", bufs=1) as cpool,
            tc.tile_pool(name="xg", bufs=2) as xgpool,
            tc.tile_pool(name="oh", bufs=2) as ohpool,
            tc.tile_pool(name="hts", bufs=3) as htspool,
            tc.tile_pool(name="yst", bufs=2) as ystpool,
            tc.tile_pool(name="psh", bufs=sblk, space="PSUM") as phpool,
            tc.tile_pool(name="psy", bufs=2, space="PSUM") as pypool,
        ):
            idx_s = cpool.tile([P, 8 * T], I16)
            nc.sync.dma_start(out=idx_s[:], in_=idx_d[:])
            dloc_s = cpool.tile([P, T], F32)
            nc.sync.dma_start(out=dloc_s[:], in_=dloc_d[:])
            val_s = cpool.tile([P, T], F32)
            nc.sync.dma_start(out=val_s[:], in_=val_d[:])
            wt_s = cpool.tile([P, D], F32)
            nc.sync.dma_start(out=wt_s[:], in_=wt_d[:])
            bb_s = cpool.tile([P, D], F32)
            nc.sync.dma_start(out=bb_s[:], in_=bb_d[:])
            iota_s = cpool.tile([P, P], F32)
            nc.sync.dma_start(out=iota_s[:], in_=iota_d[:])

            h_psum = {}
            ystage = None
            yst_base = 0
            for q, t0, R in runs:
                xg = xgpool.tile([P, R * D], F32, tag="xg")
                nc.gpsimd.dma_gather(
                    out_ap=xg[:, : R * D].rearrange("p (g f) -> p g f", f=D),
                    in_ap=x_d[int(ch[q]) : int(ch[q + 1]), :],
                    idxs_ap=idx_s[:, t0 * 8 : (t0 + R) * 8],
                    num_idxs=R * P,
                    num_idxs_reg=R * P,
                    elem_size=D,
                    single_packet=False,
                )
                oh = ohpool.tile([P, R * D], F32, tag="oh")
                for j in range(R):
                    t = t0 + j
                    b, _ = batches[t]
                    nc.vector.tensor_scalar(
                        out=oh[:, j * D : (j + 1) * D],
                        in0=iota_s[:],
                        scalar1=dloc_s[:, t : t + 1],
                        scalar2=val_s[:, t : t + 1],
                        op0=mybir.AluOpType.is_equal,
                        op1=mybir.AluOpType.mult,
                    )
                    if t == first[b]:
                        h_psum[b] = phpool.tile([P, D], F32, tag="hp", name=f"hp{b}")
                    nc.tensor.matmul(
                        out=h_psum[b][:],
                        lhsT=xg[:, j * D : (j + 1) * D],
                        rhs=oh[:, j * D : (j + 1) * D],
                        start=(t == first[b]),
                        stop=(t == last[b]),
                    )
                    if t == last[b]:
                        hts = htspool.tile([P, D], F32, tag="hts")
                        nc.scalar.activation(
                            out=hts[:],
                            in_=h_psum[b][:],
                            func=mybir.ActivationFunctionType.Copy,
                        )
                        del h_psum[b]
                        yps = pypool.tile([P, D], F32, tag="yp")
                        nc.tensor.matmul(
                            out=yps[:],
                            lhsT=hts[:],
                            rhs=wt_s[:],
                            start=True,
                            stop=True,
                        )
                        if b % sblk == 0:
                            ystage = ystpool.tile([P, sblk * D], F32, tag="yst")
                            yst_base = b
                        g = b - yst_base
                        nc.vector.tensor_tensor(
                            out=ystage[:, g * D : (g + 1) * D],
                            in0=yps[:],
                            in1=bb_s[:],
                            op=mybir.AluOpType.add,
                        )
                        if b == nb - 1 or g == sblk - 1:
                            ns = g + 1
                            rows = y_d[yst_base * P : (yst_base + ns) * P, :]
                            nc.sync.dma_start(
                                out=rows.rearrange("(g p) f -> p g f", p=P),
                                in_=ystage[:, : ns * D].rearrange(
                                    "p (g f) -> p g f", f=D
                                ),
                            )
    nc.finalize()
    return nc


def _make_in_maps(inputs, n_cores=N_CORES, npc=NPC, chunk=CHUNK, sblk=SBLK):
    X = np.ascontiguousarray(np.asarray(inputs["X"], dtype=np.float32))
    A_rows = np.asarray(inputs["A_rows"], dtype=np.int64)
    A_cols = np.asarray(inputs["A_cols"], dtype=np.int64)
    A_vals = np.asarray(inputs["A_vals"], dtype=np.float32)
    W = np.asarray(inputs["W"], dtype=np.float32)
    bias = np.asarray(inputs["b"], dtype=np.float32)

    n_nodes = X.shape[0]
    ch = _chunk_bounds(n_nodes, chunk)
    metas, sched, nb, _ = _prep(A_rows, A_cols, A_vals, n_cores, npc, ch, sblk)
    wt = np.ascontiguousarray(W.T)
    bb = np.broadcast_to(bias[None, :], (P, D)).copy()
    iota = np.broadcast_to(np.arange(P, dtype=np.float32)[None, :], (P, P)).copy()
    in_maps = []
    for idx_all, dloc_t, val_t in metas:
        in_maps.append(
            {
                "X": X,
                "idx": idx_all,
                "dloc": dloc_t,
                "val": val_t,
                "wt": wt,
                "bb": bb,
                "iota": iota,
            }
        )
    return in_maps, ch, sched, nb


def _run(inputs, trace=False, **kw):
    in_maps, ch, sched, nb = _make_in_maps(inputs)
    nc = _build_program(np.asarray(inputs["X"]).shape[0], ch, sched, nb, SBLK)
    res = run_bass_kernel_spmd(nc, in_maps, list(range(N_CORES)), trace=trace, **kw)
    out = np.concatenate([res.results[c]["y"][:NPC] for c in range(N_CORES)], axis=0)
    return out, res


def kernel(**inputs):
    return _run(inputs, trace=False)[0]
